# revision 40
# baseline (speedup 1.0000x reference)
"""Additive (Bahdanau) attention on 8 Trainium2 NeuronCores.

Reference computation (per batch row b):
    q_proj = query @ W1                                  # (H,)
    k_proj = keys @ W2                                   # (S, H)
    scores = tanh(q_proj + k_proj) @ v                   # (S,)
    scores = where(mask == 0, -1e9, scores)
    attn   = softmax(scores)                             # (S,)
    ctx    = attn @ values                               # (H,)
Returns (context (B, H), attn (B, S)).

Sharding: pure data-parallel over batch: 16 rows over 8 cores -> 2 rows per
core, weights replicated, no collectives.

Per-core pipeline (B_PC=2, S=2048, H=1024), matmuls in bf16 (4x faster than
f32 on the PE; end-to-end rel err ~2e-3 vs the 2e-2 gate):

 - All bulk loads are plain f32 on the two parallel HWDGE rings (SP ring:
   keys + W2; ACT ring: W1 + values) -- SWDGE cast-DMAs measured only
   ~200 GB/s vs ~360+ for plain HWDGE, which made DMA the bottleneck.
   f32->bf16 casts run on the otherwise-idle GPSIMD engine (keys, values)
   and DVE (W2) instead.
 - Startup: dummy-matmul HAM warmup + f32 q_projT (dc-major contiguous
   PSUM groups; interleaved accumulation groups on one PSUM tile corrupt
   results) fill the PE while keys chunk 0 and W2 stream in.
 - Per (b, s-chunk of 512): PE-transposes 128x128 keys tiles into keysT
   (the PE contracts over the partition dim), interleaved into the
   previous chunk's matmul stream so the HAM clock never sees an idle
   window. Then per d-chunk: 8-matmul PSUM accumulation, scalar-engine
   tanh with fused q_projT bias (drains PSUM), and a skinny v^T matmul
   accumulating scores. v-matmuls and the score epilogue are emitted late
   (software pipelining) so the strict-FIFO PE queue never waits on the
   scalar/vector engines.
 - Softmax without max-subtraction: |scores| <= ||v||_1 <= 32 so f32 exp
   cannot overflow; masked entries become exact zeros via a 0/1 f32 mask
   multiply (mask built with copy_predicated, no int->float cast). Per
   chunk: exp straight from PSUM (scalar engine), mask-mult + running Z
   (DVE), PE transpose of unnormalized weights to per-partition columns,
   context matmuls attn_col^T @ values (bf16) accumulated in PSUM and
   summed in SBUF. Normalization by 1/Z happens once per batch on the
   tiny (1, S) / (1, H) tensors -- no serial softmax->context tail.
"""

import numpy as np

H = 1024
S = 2048
B = 16
NCORES = 8
B_PC = B // NCORES  # batch rows per core
SC = 512            # seq chunk for the main matmul
NSC = S // SC       # 4
NHC = H // 128      # 8 h-chunks (contraction)
NDC = H // 128      # 8 d-chunks (output hidden)
N_WARM = 24

_CACHE = {}


def _build():
    import concourse.bass as bass
    import concourse.tile as tile
    from concourse import bacc, mybir
    from contextlib import ExitStack

    f32 = mybir.dt.float32
    bf16 = mybir.dt.bfloat16
    i32 = mybir.dt.int32
    Tanh = mybir.ActivationFunctionType.Tanh
    Exp = mybir.ActivationFunctionType.Exp
    AX = mybir.AxisListType.X

    nc = bacc.Bacc("TRN2", target_bir_lowering=False, debug=False)

    keys_e = nc.declare_dram_parameter("keys", [B_PC, S, H], f32, isOutput=False)
    values_e = nc.declare_dram_parameter("values", [B_PC, S, H], f32, isOutput=False)
    query_e = nc.declare_dram_parameter("query", [B_PC, H], f32, isOutput=False)
    mask_e = nc.declare_dram_parameter("mask", [B_PC, S], i32, isOutput=False)
    w1_e = nc.declare_dram_parameter("W1", [H, H], f32, isOutput=False)
    w2_e = nc.declare_dram_parameter("W2", [H, H], f32, isOutput=False)
    v_e = nc.declare_dram_parameter("v", [H], f32, isOutput=False)
    id_e = nc.declare_dram_parameter("ident", [128, 128], bf16, isOutput=False)
    octx_e = nc.declare_dram_parameter("out_ctx", [B_PC, H], f32, isOutput=True)
    oattn_e = nc.declare_dram_parameter("out_attn", [B_PC, S], f32, isOutput=True)

    with tile.TileContext(nc) as tc, ExitStack() as ctx:
        persist = ctx.enter_context(tc.tile_pool(name="persist", bufs=1))
        knf_pool = ctx.enter_context(tc.tile_pool(name="knf", bufs=3))
        kn_pool = ctx.enter_context(tc.tile_pool(name="kn", bufs=2))
        vf_pool = ctx.enter_context(tc.tile_pool(name="vf", bufs=4))
        kt_pool = ctx.enter_context(tc.tile_pool(name="kt", bufs=2))
        th_pool = ctx.enter_context(tc.tile_pool(name="th", bufs=4))
        w1_pool = ctx.enter_context(tc.tile_pool(name="w1p", bufs=3))
        val_pool = ctx.enter_context(tc.tile_pool(name="val", bufs=6))
        tmp_pool = ctx.enter_context(tc.tile_pool(name="tmp", bufs=2))
        mk_pool = ctx.enter_context(tc.tile_pool(name="mk", bufs=1))
        ps_k = ctx.enter_context(tc.tile_pool(name="ps_k", bufs=2, space="PSUM"))
        ps_tr = ctx.enter_context(tc.tile_pool(name="ps_tr", bufs=2, space="PSUM"))
        ps_sc = ctx.enter_context(tc.tile_pool(name="ps_sc", bufs=2, space="PSUM"))
        ps_ctx = ctx.enter_context(tc.tile_pool(name="ps_ctx", bufs=1, space="PSUM"))

        # ---- PE warmup: dense dummy matmuls while the first DMAs land ----
        wtile = persist.tile([128, 128], bf16, tag="wtile")
        nc.vector.memset(wtile, 0.5)
        for i in range(N_WARM):
            wps = ps_k.tile([128, 128], f32, tag="kproj", name=f"wps{i}")
            nc.tensor.matmul(wps, wtile, wtile, start=True, stop=True)

        # ---- keys pipeline: f32 on the SP HWDGE ring, bf16 cast on DVE.
        kn_f32 = {}
        kn_bf = {}

        def load_knf(b, sc, ring=None):
            t = knf_pool.tile([128, NSC, H], f32, tag="knf", name=f"knf{b}{sc}")
            (ring or nc.sync).dma_start(
                out=t,
                in_=keys_e[b, sc * SC:(sc + 1) * SC, :].rearrange(
                    "(ss p) h -> p ss h", p=128
                ),
            )
            kn_f32[(b, sc)] = t

        def cast_kn(b, sc):
            t = kn_pool.tile([128, NSC, H], bf16, tag="keysN", name=f"kn{b}{sc}")
            nc.vector.tensor_copy(t, kn_f32.pop((b, sc)))
            kn_bf[(b, sc)] = t


        # W2 f32 via knf-pool staging tiles -> DVE cast to bf16, interleaved
        # with the first two keys chunks on the SP ring
        w2_sb = persist.tile([128, NHC, H], bf16, tag="w2")

        Copy = mybir.ActivationFunctionType.Copy

        def load_w2_half(half):
            w2f = knf_pool.tile([128, NSC, H], f32, tag="knf", name=f"w2f{half}")
            nc.sync.dma_start(
                out=w2f,
                in_=w2_e[half * 512:(half + 1) * 512, :].rearrange(
                    "(hc p) d -> p hc d", p=128
                ),
            )
            nc.scalar.activation(
                out=w2_sb[:, half * 4:(half + 1) * 4, :], in_=w2f, func=Copy,
            )

        ident = persist.tile([128, 128], bf16, tag="ident")
        nc.sync.dma_start(out=ident, in_=id_e[:, :])
        qT_sb = persist.tile([128, NHC, B_PC], f32, tag="qT")
        for b in range(B_PC):
            nc.sync.dma_start(
                out=qT_sb[:, :, b],
                in_=query_e[b, :].rearrange("(hc p) -> p hc", p=128),
            )
        load_w2_half(0)
        load_knf(0, 0)
        load_w2_half(1)
        cast_kn(0, 0)
        load_knf(0, 1)

        vraw = persist.tile([128, NDC], f32, tag="vraw")
        nc.sync.dma_start(out=vraw, in_=v_e[:].rearrange("(dc p) -> p dc", p=128))
        v_sb = persist.tile([128, NDC], bf16, tag="v")
        nc.vector.tensor_copy(v_sb, vraw)

        ones_row = persist.tile([1, SC], f32, tag="ones_row")
        nc.vector.memset(ones_row, 1.0)
        one_bf = persist.tile([1, 1], bf16, tag="one_bf")
        nc.vector.memset(one_bf, 1.0)

        # ---- q_projT in f32 (W1 column-slices on the ACT HWDGE ring),
        # dc-major so each PSUM accumulation group is contiguous. Only the
        # first few d-chunks are emitted here; the rest interleave into
        # chunk 0's dc-loop so late W1 arrivals (competing with the 8MB
        # keys/W2 stream) never head-of-line-block the PE FIFO. ----
        qp_sb = persist.tile([128, NDC, B_PC], f32, tag="qp")

        w1c_tiles = {}

        def load_w1c(dc):
            w1c = w1_pool.tile([128, NHC, 128], f32, tag="w1c", name=f"w1c{dc}")
            nc.scalar.dma_start(
                out=w1c,
                in_=w1_e[:, dc * 128:(dc + 1) * 128].rearrange(
                    "(hc p) d -> p hc d", p=128
                ),
            )
            w1c_tiles[dc] = w1c

        def emit_qproj(dc):
            w1c = w1c_tiles.pop(dc)
            qp_ps = ps_sc.tile([128, B_PC], f32, tag="sc", name=f"qpps{dc}")
            for hc in range(NHC):
                nc.tensor.matmul(
                    qp_ps, w1c[:, hc, :], qT_sb[:, hc, :],
                    start=(hc == 0), stop=(hc == NHC - 1),
                )
            nc.vector.tensor_copy(qp_sb[:, dc, :], qp_ps)

        for dc in range(3):
            load_w1c(dc)
        for dc in range(2):
            emit_qproj(dc)

        # ---- additive masks: madd = (mask - 1) * 1e9 (0 where mask=1,
        # -1e9 where mask=0); built with copy_predicated, no int->f32 cast.
        # Added into the score PSUM via a K=1 rank-1 matmul so exp() gets
        # pre-masked scores and no per-chunk vector work is needed. ----
        madd_sb = []
        for b in range(B_PC):
            mi = mk_pool.tile([1, S], i32, tag="maski", name=f"maski{b}")
            nc.sync.dma_start(out=mi, in_=mask_e[b, :].rearrange("(o s) -> o s", o=1))
            mf = persist.tile([1, S], f32, tag=f"madd{b}", name=f"madd{b}")
            nc.vector.memset(mf, 0.0)
            for q in range(NSC):
                nc.vector.copy_predicated(
                    mf[:, q * SC:(q + 1) * SC], mi[:, q * SC:(q + 1) * SC], ones_row
                )
            nc.vector.tensor_scalar(
                mf, mf, 1.0, 1.0e9,
                op0=mybir.AluOpType.subtract, op1=mybir.AluOpType.mult,
            )
            mb = persist.tile([1, S], bf16, tag=f"maddb{b}", name=f"maddb{b}")
            nc.vector.tensor_copy(mb, mf)
            madd_sb.append(mb)

        attn_un = [
            persist.tile([1, S], f32, tag=f"attnun{b}", name=f"attnun{b}")
            for b in range(B_PC)
        ]
        zpart = persist.tile([1, B_PC * NSC], f32, tag="zpart")
        at_sb = [
            persist.tile([128, S // 128], bf16, tag=f"at{b}", name=f"at{b}")
            for b in range(B_PC)
        ]
        ctx_ps_cur = [None]

        chunks = [(b, sc) for b in range(B_PC) for sc in range(NSC)]

        # Deferred emission (software pipelining of the strict-FIFO PE queue).
        pend_v = None
        pend_epi_a = None
        pend_epi_b = None

        def make_epilogue_a(ci, b, sc, sc_ps):
            def emit():
                sl = slice(sc * SC, (sc + 1) * SC)
                nc.scalar.activation(
                    out=attn_un[b][:, sl], in_=sc_ps, func=Exp, scale=1.0,
                    accum_out=zpart[:, b * NSC + sc:b * NSC + sc + 1],
                )
            return emit

        def make_epilogue_b(ci, b, sc, vals):
            def emit():
                at_ps = ps_k.tile([128, NSC], f32, tag="kproj", name=f"atps{ci}")
                for ch4 in range(NSC):
                    nc.tensor.transpose(
                        at_ps[:, ch4:ch4 + 1],
                        attn_un[b][:, sc * SC + ch4 * 128: sc * SC + (ch4 + 1) * 128],
                        ones_row[0:1, 0:1],
                    )
                nc.vector.tensor_copy(at_sb[b][:, sc * NSC:(sc + 1) * NSC], at_ps)
                if sc == 0:
                    ctx_ps_cur[0] = [
                        ps_ctx.tile([1, SC], f32, tag=f"ctx{hf}", name=f"ctxps{b}{hf}")
                        for hf in range(2)
                    ]
                ctx_ps = ctx_ps_cur[0]
                for ch4 in range(NSC):
                    ch = sc * NSC + ch4
                    for hf in range(2):
                        nc.tensor.matmul(
                            ctx_ps[hf],
                            at_sb[b][:, ch:ch + 1],
                            vals[ch4][:, hf * 512:(hf + 1) * 512],
                            start=(ch == 0),
                            stop=(ch == S // 128 - 1),
                        )
                if sc == NSC - 1:
                    zt = tmp_pool.tile([1, 1], f32, tag="zt", name=f"zt{b}")
                    nc.vector.reduce_sum(
                        zt, zpart[:, b * NSC:(b + 1) * NSC], axis=AX
                    )
                    rz = tmp_pool.tile([1, 1], f32, tag="rz", name=f"rz{b}")
                    nc.vector.reciprocal(rz, zt)
                    Copy2 = mybir.ActivationFunctionType.Copy
                    for osc in range(NSC):
                        osl = slice(osc * SC, (osc + 1) * SC)
                        ao = tmp_pool.tile(
                            [1, SC], f32, tag="attno", name=f"ao{b}{osc}"
                        )
                        if osc % 2 == 0:
                            nc.scalar.activation(
                                out=ao, in_=attn_un[b][:, osl],
                                func=Copy2, scale=rz,
                            )
                        else:
                            nc.vector.tensor_scalar_mul(ao, attn_un[b][:, osl], rz)
                        nc.sync.dma_start(out=oattn_e[b, osl], in_=ao)
                    ctxo = tmp_pool.tile([1, H], f32, tag="ctxo", name=f"ctxo{b}")
                    for hf in range(2):
                        nc.vector.tensor_scalar_mul(
                            ctxo[:, hf * 512:(hf + 1) * 512], ctx_ps[hf], rz
                        )
                    nc.sync.dma_start(out=octx_e[b, :], in_=ctxo)
            return emit

        def emit_transpose_hc(ci, keysN, keysT, hc):
            tr_ps = ps_tr.tile([128, SC], bf16, tag="tr", name=f"tr{ci}{hc}")
            for ss in range(NSC):
                nc.tensor.transpose(
                    tr_ps[:, ss * 128:(ss + 1) * 128],
                    keysN[:, ss, hc * 128:(hc + 1) * 128],
                    ident,
                )
            nc.vector.tensor_copy(keysT[:, hc, :], tr_ps)

        keysT_cur = None
        for ci, (b, sc) in enumerate(chunks):
            # values for this chunk: f32 on the ACT ring, bf16 cast on GPSIMD
            if ci == 0:
                if ci + 2 < len(chunks):
                    load_knf(*chunks[ci + 2])
                if ci + 3 < len(chunks):
                    load_knf(*chunks[ci + 3])
            vf_list = []
            for ch4 in range(NSC):
                ch = sc * NSC + ch4
                vf = vf_pool.tile([128, H], f32, tag="vf", name=f"vf{ci}{ch4}")
                nc.sync.dma_start(
                    out=vf, in_=values_e[b, ch * 128:(ch + 1) * 128, :]
                )
                vf_list.append(vf)
            vals_list = []

            def cast_vals(ci=ci, vf_list=vf_list, vals_list=vals_list):
                for ch4, vf in enumerate(vf_list):
                    vb = val_pool.tile(
                        [128, H], bf16, tag="valN", name=f"val{ci}{ch4}"
                    )
                    if ch4 % 2 == 0:
                        nc.vector.tensor_copy(vb, vf)
                    else:
                        nc.gpsimd.tensor_copy(vb, vf)
                    vals_list.append(vb)
            # keys prefetch: f32 loads 2-3 chunks ahead; the bf16 casts are
            # emitted inside the dc loop (after this chunk's keysT drains)
            # so a cast waiting on its DMA can't head-of-line-block the DVE
            # FIFO in front of the drains.
            if ci > 0 and ci + 3 < len(chunks):
                load_knf(*chunks[ci + 3])

            if ci == 0:
                keysN = kn_bf.pop((b, sc))
                keysT_cur = kt_pool.tile(
                    [128, NHC, SC], bf16, tag="keysT", name=f"kt{b}{sc}"
                )
                for hc in range(NHC):
                    emit_transpose_hc(ci, keysN, keysT_cur, hc)
            keysT = keysT_cur
            keysN_next = keysT_next = None

            sc_ps = ps_sc.tile([1, SC], f32, tag="sc", name=f"scps{ci}")
            prev_th = None
            for dc in range(NDC):
                k_ps = ps_k.tile([128, SC], f32, tag="kproj", name=f"kp{ci}{dc}")
                for hc in range(NHC):
                    nc.tensor.matmul(
                        k_ps,
                        w2_sb[:, hc, dc * 128:(dc + 1) * 128],
                        keysT[:, hc, :],
                        start=(hc == 0),
                        stop=(hc == NHC - 1),
                    )
                th = th_pool.tile([128, SC], bf16, tag="th", name=f"th{ci}{dc}")
                nc.scalar.activation(
                    out=th, in_=k_ps, func=Tanh,
                    bias=qp_sb[:, dc, b:b + 1], scale=1.0,
                )
                if dc == 1 and pend_v is not None:
                    pend_v()
                    pend_v = None
                if dc == 1 and ci + 1 < len(chunks) and chunks[ci + 1] in kn_f32:
                    cast_kn(*chunks[ci + 1])
                if dc == 2 and pend_epi_a is not None:
                    pend_epi_a()
                    pend_epi_a = None
                if ci == 0 and 1 <= dc <= 6:
                    if dc <= 5:
                        load_w1c(dc + 2)
                    emit_qproj(dc + 1)
                if dc == (7 if ci == 0 else 5):
                    cast_vals()
                if dc == 3 and ci + 1 < len(chunks):
                    nb, nsc = chunks[ci + 1]
                    keysN_next = kn_bf.pop((nb, nsc))
                    keysT_next = kt_pool.tile(
                        [128, NHC, SC], bf16, tag="keysT", name=f"kt{nb}{nsc}"
                    )
                if dc == 6 and pend_epi_b is not None:
                    pend_epi_b()
                    pend_epi_b = None
                if keysT_next is not None:
                    for thc in {3: [0], 4: [1, 2], 5: [3, 4],
                                6: [5, 6], 7: [7]}.get(dc, []):
                        emit_transpose_hc(ci + 1, keysN_next, keysT_next, thc)
                if dc >= 1:
                    nc.tensor.matmul(
                        sc_ps, v_sb[:, dc - 1:dc], prev_th,
                        start=(dc - 1 == 0), stop=False,
                    )
                prev_th = th

            def make_last_v(sc_ps=sc_ps, th=prev_th, b=b, sc=sc):
                def emit():
                    nc.tensor.matmul(
                        sc_ps, v_sb[:, NDC - 1:NDC], th, start=False, stop=False
                    )
                    nc.tensor.matmul(
                        sc_ps,
                        one_bf[0:1, 0:1],
                        madd_sb[b][:, sc * SC:(sc + 1) * SC],
                        start=False, stop=True,
                    )
                return emit
            pend_v = make_last_v()
            pend_epi_a = make_epilogue_a(ci, b, sc, sc_ps)
            pend_epi_b = make_epilogue_b(ci, b, sc, vals_list)
            keysT_cur = keysT_next

        pend_v()
        pend_epi_a()
        pend_epi_b()

    nc.compile()
    return nc


def _get_nc():
    if "nc" not in _CACHE:
        _CACHE["nc"] = _build()
    return _CACHE["nc"]


def _make_in_maps(inputs):
    import ml_dtypes

    q = np.asarray(inputs["query"], dtype=np.float32)
    k = np.asarray(inputs["keys"], dtype=np.float32)
    val = np.asarray(inputs["values"], dtype=np.float32)
    m = np.asarray(inputs["mask"], dtype=np.int32)
    w1 = np.ascontiguousarray(np.asarray(inputs["W1"], dtype=np.float32))
    w2 = np.ascontiguousarray(np.asarray(inputs["W2"], dtype=np.float32))
    v = np.ascontiguousarray(np.asarray(inputs["v"], dtype=np.float32))
    ident = np.eye(128, dtype=ml_dtypes.bfloat16)

    in_maps = []
    for c in range(NCORES):
        sl = slice(c * B_PC, (c + 1) * B_PC)
        in_maps.append({
            "keys": np.ascontiguousarray(k[sl]),
            "values": np.ascontiguousarray(val[sl]),
            "query": np.ascontiguousarray(q[sl]),
            "mask": np.ascontiguousarray(m[sl]),
            "W1": w1, "W2": w2, "v": v, "ident": ident,
        })
    return in_maps


def kernel(**inputs):
    import time
    from concourse.bass_utils import run_bass_kernel_spmd

    nc = _get_nc()
    in_maps = _make_in_maps(inputs)
    last_err = None
    for attempt in range(3):
        try:
            res = run_bass_kernel_spmd(nc, in_maps, list(range(NCORES)))
            break
        except Exception as e:  # transient device errors: retry
            last_err = e
            time.sleep(2.0)
    else:
        raise last_err
    ctx = np.concatenate([res.results[c]["out_ctx"] for c in range(NCORES)], axis=0)
    attn = np.concatenate([res.results[c]["out_attn"] for c in range(NCORES)], axis=0)
    return ctx, attn


# revision 41
# speedup vs baseline: 1.0283x; 1.0283x over previous
"""Additive (Bahdanau) attention on 8 Trainium2 NeuronCores.

Reference computation (per batch row b):
    q_proj = query @ W1                                  # (H,)
    k_proj = keys @ W2                                   # (S, H)
    scores = tanh(q_proj + k_proj) @ v                   # (S,)
    scores = where(mask == 0, -1e9, scores)
    attn   = softmax(scores)                             # (S,)
    ctx    = attn @ values                               # (H,)
Returns (context (B, H), attn (B, S)).

Sharding: pure data-parallel over batch: 16 rows over 8 cores -> 2 rows per
core, weights replicated, no collectives.

Per-core pipeline (B_PC=2, S=2048, H=1024), matmuls in bf16 (4x faster than
f32 on the PE; end-to-end rel err ~2e-3 vs the 2e-2 gate):

 - All bulk loads are plain f32 on the two parallel HWDGE rings (SP ring:
   keys + W2; ACT ring: W1 + values) -- SWDGE cast-DMAs measured only
   ~200 GB/s vs ~360+ for plain HWDGE, which made DMA the bottleneck.
   f32->bf16 casts run on the otherwise-idle GPSIMD engine (keys, values)
   and DVE (W2) instead.
 - Startup: dummy-matmul HAM warmup + f32 q_projT (dc-major contiguous
   PSUM groups; interleaved accumulation groups on one PSUM tile corrupt
   results) fill the PE while keys chunk 0 and W2 stream in.
 - Per (b, s-chunk of 512): PE-transposes 128x128 keys tiles into keysT
   (the PE contracts over the partition dim), interleaved into the
   previous chunk's matmul stream so the HAM clock never sees an idle
   window. Then per d-chunk: 8-matmul PSUM accumulation, scalar-engine
   tanh with fused q_projT bias (drains PSUM), and a skinny v^T matmul
   accumulating scores. v-matmuls and the score epilogue are emitted late
   (software pipelining) so the strict-FIFO PE queue never waits on the
   scalar/vector engines.
 - Softmax without max-subtraction: |scores| <= ||v||_1 <= 32 so f32 exp
   cannot overflow; masked entries become exact zeros via a 0/1 f32 mask
   multiply (mask built with copy_predicated, no int->float cast). Per
   chunk: exp straight from PSUM (scalar engine), mask-mult + running Z
   (DVE), PE transpose of unnormalized weights to per-partition columns,
   context matmuls attn_col^T @ values (bf16) accumulated in PSUM and
   summed in SBUF. Normalization by 1/Z happens once per batch on the
   tiny (1, S) / (1, H) tensors -- no serial softmax->context tail.
"""

import numpy as np

H = 1024
S = 2048
B = 16
NCORES = 8
B_PC = B // NCORES  # batch rows per core
SC = 512            # seq chunk for the main matmul
NSC = S // SC       # 4
NHC = H // 128      # 8 h-chunks (contraction)
NDC = H // 128      # 8 d-chunks (output hidden)
N_WARM = 10

_CACHE = {}


def _build():
    import concourse.bass as bass
    import concourse.tile as tile
    from concourse import bacc, mybir
    from contextlib import ExitStack

    f32 = mybir.dt.float32
    bf16 = mybir.dt.bfloat16
    i32 = mybir.dt.int32
    Tanh = mybir.ActivationFunctionType.Tanh
    Exp = mybir.ActivationFunctionType.Exp
    AX = mybir.AxisListType.X

    nc = bacc.Bacc("TRN2", target_bir_lowering=False, debug=False)

    keys_e = nc.declare_dram_parameter("keys", [B_PC, S, H], f32, isOutput=False)
    values_e = nc.declare_dram_parameter("values", [B_PC, S, H], f32, isOutput=False)
    query_e = nc.declare_dram_parameter("query", [B_PC, H], f32, isOutput=False)
    mask_e = nc.declare_dram_parameter("mask", [B_PC, S], i32, isOutput=False)
    w1_e = nc.declare_dram_parameter("W1", [H, H], f32, isOutput=False)
    w2_e = nc.declare_dram_parameter("W2", [H, H], f32, isOutput=False)
    v_e = nc.declare_dram_parameter("v", [H], f32, isOutput=False)
    id_e = nc.declare_dram_parameter("ident", [128, 128], bf16, isOutput=False)
    idf_e = nc.declare_dram_parameter("identf", [8, 8], f32, isOutput=False)
    octx_e = nc.declare_dram_parameter("out_ctx", [B_PC, H], f32, isOutput=True)
    oattn_e = nc.declare_dram_parameter("out_attn", [B_PC, S], f32, isOutput=True)

    with tile.TileContext(nc) as tc, ExitStack() as ctx:
        persist = ctx.enter_context(tc.tile_pool(name="persist", bufs=1))
        knf_pool = ctx.enter_context(tc.tile_pool(name="knf", bufs=3))
        kn_pool = ctx.enter_context(tc.tile_pool(name="kn", bufs=2))
        vf_pool = ctx.enter_context(tc.tile_pool(name="vf", bufs=4))
        kt_pool = ctx.enter_context(tc.tile_pool(name="kt", bufs=2))
        th_pool = ctx.enter_context(tc.tile_pool(name="th", bufs=4))
        w1_pool = ctx.enter_context(tc.tile_pool(name="w1p", bufs=3))
        val_pool = ctx.enter_context(tc.tile_pool(name="val", bufs=6))
        tmp_pool = ctx.enter_context(tc.tile_pool(name="tmp", bufs=2))
        mk_pool = ctx.enter_context(tc.tile_pool(name="mk", bufs=1))
        ps_k = ctx.enter_context(tc.tile_pool(name="ps_k", bufs=2, space="PSUM"))
        ps_tr = ctx.enter_context(tc.tile_pool(name="ps_tr", bufs=2, space="PSUM"))
        ps_sc = ctx.enter_context(tc.tile_pool(name="ps_sc", bufs=2, space="PSUM"))
        ps_ctx = ctx.enter_context(tc.tile_pool(name="ps_ctx", bufs=1, space="PSUM"))

        # ---- PE warmup: dense dummy matmuls while the first DMAs land ----
        wtile = persist.tile([128, 128], bf16, tag="wtile")
        nc.vector.memset(wtile, 0.5)
        for i in range(N_WARM):
            wps = ps_k.tile([128, 128], f32, tag="kproj", name=f"wps{i}")
            nc.tensor.matmul(wps, wtile, wtile, start=True, stop=True)

        # ---- keys pipeline: f32 on the SP HWDGE ring, bf16 cast on DVE.
        kn_f32 = {}
        kn_bf = {}

        def load_knf(b, sc, ring=None):
            t = knf_pool.tile([128, NSC, H], f32, tag="knf", name=f"knf{b}{sc}")
            (ring or nc.sync).dma_start(
                out=t,
                in_=keys_e[b, sc * SC:(sc + 1) * SC, :].rearrange(
                    "(ss p) h -> p ss h", p=128
                ),
            )
            kn_f32[(b, sc)] = t

        def cast_kn(b, sc):
            t = kn_pool.tile([128, NSC, H], bf16, tag="keysN", name=f"kn{b}{sc}")
            nc.vector.tensor_copy(t, kn_f32.pop((b, sc)))
            kn_bf[(b, sc)] = t


        # W2 f32 via knf-pool staging tiles -> DVE cast to bf16, interleaved
        # with the first two keys chunks on the SP ring
        w2_sb = persist.tile([128, NHC, H], bf16, tag="w2")

        Copy = mybir.ActivationFunctionType.Copy

        def load_w2_half(half):
            w2f = knf_pool.tile([128, NSC, H], f32, tag="knf", name=f"w2f{half}")
            nc.sync.dma_start(
                out=w2f,
                in_=w2_e[half * 512:(half + 1) * 512, :].rearrange(
                    "(hc p) d -> p hc d", p=128
                ),
            )
            nc.scalar.activation(
                out=w2_sb[:, half * 4:(half + 1) * 4, :], in_=w2f, func=Copy,
            )

        ident = persist.tile([128, 128], bf16, tag="ident")
        nc.sync.dma_start(out=ident, in_=id_e[:, :])
        identf = persist.tile([8, 8], f32, tag="identf")
        nc.sync.dma_start(out=identf, in_=idf_e[:, :])
        qT_sb = persist.tile([128, NHC, B_PC], f32, tag="qT")
        for b in range(B_PC):
            nc.sync.dma_start(
                out=qT_sb[:, :, b],
                in_=query_e[b, :].rearrange("(hc p) -> p hc", p=128),
            )
        load_w2_half(0)
        load_knf(0, 0)
        load_w2_half(1)
        cast_kn(0, 0)
        load_knf(0, 1)

        vraw = persist.tile([128, NDC], f32, tag="vraw")
        nc.sync.dma_start(out=vraw, in_=v_e[:].rearrange("(dc p) -> p dc", p=128))
        v_sb = persist.tile([128, NDC], bf16, tag="v")
        nc.vector.tensor_copy(v_sb, vraw)

        ones_row = persist.tile([1, SC], f32, tag="ones_row")
        nc.vector.memset(ones_row, 1.0)
        one_bf = persist.tile([1, 1], bf16, tag="one_bf")
        nc.vector.memset(one_bf, 1.0)

        # ---- q_proj in natural layout: lhsT = qT chunk (stationary),
        # rhs = W1 ROW-chunks (contiguous 4KB rows -> cheap DMA descriptors;
        # column slices cost ~4.3us of sequencer issue time each). The tiny
        # (2, 1024) result is PE-transposed to the per-partition bias layout.
        # f32 matmuls double as the HAM warmup. ----
        qp_sb = persist.tile([128, NDC, B_PC], f32, tag="qp")
        qpn_ps = [
            ps_sc.tile([B_PC, 512], f32, tag="sc", name=f"qpn{hf}")
            for hf in range(2)
        ]
        for hc in range(NHC):
            w1r = w1_pool.tile([128, H], f32, tag="w1c", name=f"w1r{hc}")
            nc.scalar.dma_start(out=w1r, in_=w1_e[hc * 128:(hc + 1) * 128, :])
            for hf in range(2):
                nc.tensor.matmul(
                    qpn_ps[hf], qT_sb[:, hc, :],
                    w1r[:, hf * 512:(hf + 1) * 512],
                    start=(hc == 0), stop=(hc == NHC - 1),
                )
        qp_nat = persist.tile([B_PC, H], f32, tag="qpnat")
        for hf in range(2):
            nc.vector.tensor_copy(qp_nat[:, hf * 512:(hf + 1) * 512], qpn_ps[hf])
        qpt_ps = ps_sc.tile([128, NDC * B_PC], f32, tag="sc", name="qpt")
        for dc in range(NDC):
            nc.tensor.transpose(
                qpt_ps[:, dc * B_PC:(dc + 1) * B_PC],
                qp_nat[:, dc * 128:(dc + 1) * 128],
                identf[0:B_PC, 0:B_PC],
            )
        nc.vector.tensor_copy(qp_sb, qpt_ps)

        # ---- additive masks: madd = (mask - 1) * 1e9 (0 where mask=1,
        # -1e9 where mask=0); built with copy_predicated, no int->f32 cast.
        # Added into the score PSUM via a K=1 rank-1 matmul so exp() gets
        # pre-masked scores and no per-chunk vector work is needed. ----
        madd_sb = []
        for b in range(B_PC):
            mi = mk_pool.tile([1, S], i32, tag="maski", name=f"maski{b}")
            nc.sync.dma_start(out=mi, in_=mask_e[b, :].rearrange("(o s) -> o s", o=1))
            mf = persist.tile([1, S], f32, tag=f"madd{b}", name=f"madd{b}")
            nc.vector.memset(mf, 0.0)
            for q in range(NSC):
                nc.vector.copy_predicated(
                    mf[:, q * SC:(q + 1) * SC], mi[:, q * SC:(q + 1) * SC], ones_row
                )
            nc.vector.tensor_scalar(
                mf, mf, 1.0, 1.0e9,
                op0=mybir.AluOpType.subtract, op1=mybir.AluOpType.mult,
            )
            mb = persist.tile([1, S], bf16, tag=f"maddb{b}", name=f"maddb{b}")
            nc.vector.tensor_copy(mb, mf)
            madd_sb.append(mb)

        attn_un = [
            persist.tile([1, S], f32, tag=f"attnun{b}", name=f"attnun{b}")
            for b in range(B_PC)
        ]
        zpart = persist.tile([1, B_PC * NSC], f32, tag="zpart")
        at_sb = [
            persist.tile([128, S // 128], bf16, tag=f"at{b}", name=f"at{b}")
            for b in range(B_PC)
        ]
        ctx_ps_cur = [None]

        chunks = [(b, sc) for b in range(B_PC) for sc in range(NSC)]

        # Deferred emission (software pipelining of the strict-FIFO PE queue).
        pend_v = None
        pend_epi_a = None
        pend_epi_b = None

        def make_epilogue_a(ci, b, sc, sc_ps):
            def emit():
                sl = slice(sc * SC, (sc + 1) * SC)
                nc.scalar.activation(
                    out=attn_un[b][:, sl], in_=sc_ps, func=Exp, scale=1.0,
                    accum_out=zpart[:, b * NSC + sc:b * NSC + sc + 1],
                )
            return emit

        def make_epilogue_b(ci, b, sc, vals):
            def emit():
                at_ps = ps_k.tile([128, NSC], f32, tag="kproj", name=f"atps{ci}")
                for ch4 in range(NSC):
                    nc.tensor.transpose(
                        at_ps[:, ch4:ch4 + 1],
                        attn_un[b][:, sc * SC + ch4 * 128: sc * SC + (ch4 + 1) * 128],
                        ones_row[0:1, 0:1],
                    )
                nc.vector.tensor_copy(at_sb[b][:, sc * NSC:(sc + 1) * NSC], at_ps)
                if sc == 0:
                    ctx_ps_cur[0] = [
                        ps_ctx.tile([1, SC], f32, tag=f"ctx{hf}", name=f"ctxps{b}{hf}")
                        for hf in range(2)
                    ]
                ctx_ps = ctx_ps_cur[0]
                for ch4 in range(NSC):
                    ch = sc * NSC + ch4
                    for hf in range(2):
                        nc.tensor.matmul(
                            ctx_ps[hf],
                            at_sb[b][:, ch:ch + 1],
                            vals[ch4][:, hf * 512:(hf + 1) * 512],
                            start=(ch == 0),
                            stop=(ch == S // 128 - 1),
                        )
                if sc == NSC - 1:
                    zt = tmp_pool.tile([1, 1], f32, tag="zt", name=f"zt{b}")
                    nc.vector.reduce_sum(
                        zt, zpart[:, b * NSC:(b + 1) * NSC], axis=AX
                    )
                    rz = tmp_pool.tile([1, 1], f32, tag="rz", name=f"rz{b}")
                    nc.vector.reciprocal(rz, zt)
                    Copy2 = mybir.ActivationFunctionType.Copy
                    for osc in range(NSC):
                        osl = slice(osc * SC, (osc + 1) * SC)
                        ao = tmp_pool.tile(
                            [1, SC], f32, tag="attno", name=f"ao{b}{osc}"
                        )
                        if osc % 2 == 0:
                            nc.scalar.activation(
                                out=ao, in_=attn_un[b][:, osl],
                                func=Copy2, scale=rz,
                            )
                        else:
                            nc.vector.tensor_scalar_mul(ao, attn_un[b][:, osl], rz)
                        nc.sync.dma_start(out=oattn_e[b, osl], in_=ao)
                    ctxo = tmp_pool.tile([1, H], f32, tag="ctxo", name=f"ctxo{b}")
                    for hf in range(2):
                        nc.vector.tensor_scalar_mul(
                            ctxo[:, hf * 512:(hf + 1) * 512], ctx_ps[hf], rz
                        )
                    nc.sync.dma_start(out=octx_e[b, :], in_=ctxo)
            return emit

        def emit_transpose_hc(ci, keysN, keysT, hc):
            tr_ps = ps_tr.tile([128, SC], bf16, tag="tr", name=f"tr{ci}{hc}")
            for ss in range(NSC):
                nc.tensor.transpose(
                    tr_ps[:, ss * 128:(ss + 1) * 128],
                    keysN[:, ss, hc * 128:(hc + 1) * 128],
                    ident,
                )
            nc.vector.tensor_copy(keysT[:, hc, :], tr_ps)

        keysT_cur = None
        for ci, (b, sc) in enumerate(chunks):
            # values for this chunk: f32 on the ACT ring, bf16 cast on GPSIMD
            if ci == 0:
                if ci + 2 < len(chunks):
                    load_knf(*chunks[ci + 2])
                if ci + 3 < len(chunks):
                    load_knf(*chunks[ci + 3])
            vf_list = []
            for ch4 in range(NSC):
                ch = sc * NSC + ch4
                vf = vf_pool.tile([128, H], f32, tag="vf", name=f"vf{ci}{ch4}")
                nc.sync.dma_start(
                    out=vf, in_=values_e[b, ch * 128:(ch + 1) * 128, :]
                )
                vf_list.append(vf)
            vals_list = []

            def cast_vals(ci=ci, vf_list=vf_list, vals_list=vals_list):
                for ch4, vf in enumerate(vf_list):
                    vb = val_pool.tile(
                        [128, H], bf16, tag="valN", name=f"val{ci}{ch4}"
                    )
                    if ch4 % 2 == 0:
                        nc.vector.tensor_copy(vb, vf)
                    else:
                        nc.gpsimd.tensor_copy(vb, vf)
                    vals_list.append(vb)
            # keys prefetch: f32 loads 2-3 chunks ahead; the bf16 casts are
            # emitted inside the dc loop (after this chunk's keysT drains)
            # so a cast waiting on its DMA can't head-of-line-block the DVE
            # FIFO in front of the drains.
            if ci > 0 and ci + 3 < len(chunks):
                load_knf(*chunks[ci + 3])

            if ci == 0:
                keysN = kn_bf.pop((b, sc))
                keysT_cur = kt_pool.tile(
                    [128, NHC, SC], bf16, tag="keysT", name=f"kt{b}{sc}"
                )
                for hc in range(NHC):
                    emit_transpose_hc(ci, keysN, keysT_cur, hc)
            keysT = keysT_cur
            keysN_next = keysT_next = None

            sc_ps = ps_sc.tile([1, SC], f32, tag="sc", name=f"scps{ci}")
            prev_th = None
            for dc in range(NDC):
                k_ps = ps_k.tile([128, SC], f32, tag="kproj", name=f"kp{ci}{dc}")
                for hc in range(NHC):
                    nc.tensor.matmul(
                        k_ps,
                        w2_sb[:, hc, dc * 128:(dc + 1) * 128],
                        keysT[:, hc, :],
                        start=(hc == 0),
                        stop=(hc == NHC - 1),
                    )
                th = th_pool.tile([128, SC], bf16, tag="th", name=f"th{ci}{dc}")
                nc.scalar.activation(
                    out=th, in_=k_ps, func=Tanh,
                    bias=qp_sb[:, dc, b:b + 1], scale=1.0,
                )
                if dc == 1 and pend_v is not None:
                    pend_v()
                    pend_v = None
                if dc == 1 and ci + 1 < len(chunks) and chunks[ci + 1] in kn_f32:
                    cast_kn(*chunks[ci + 1])
                if dc == 2 and pend_epi_a is not None:
                    pend_epi_a()
                    pend_epi_a = None
                if dc == (7 if ci == 0 else 5):
                    cast_vals()
                if dc == 3 and ci + 1 < len(chunks):
                    nb, nsc = chunks[ci + 1]
                    keysN_next = kn_bf.pop((nb, nsc))
                    keysT_next = kt_pool.tile(
                        [128, NHC, SC], bf16, tag="keysT", name=f"kt{nb}{nsc}"
                    )
                if dc == 6 and pend_epi_b is not None:
                    pend_epi_b()
                    pend_epi_b = None
                if keysT_next is not None:
                    for thc in {3: [0], 4: [1, 2], 5: [3, 4],
                                6: [5, 6], 7: [7]}.get(dc, []):
                        emit_transpose_hc(ci + 1, keysN_next, keysT_next, thc)
                if dc >= 1:
                    nc.tensor.matmul(
                        sc_ps, v_sb[:, dc - 1:dc], prev_th,
                        start=(dc - 1 == 0), stop=False,
                    )
                prev_th = th

            def make_last_v(sc_ps=sc_ps, th=prev_th, b=b, sc=sc):
                def emit():
                    nc.tensor.matmul(
                        sc_ps, v_sb[:, NDC - 1:NDC], th, start=False, stop=False
                    )
                    nc.tensor.matmul(
                        sc_ps,
                        one_bf[0:1, 0:1],
                        madd_sb[b][:, sc * SC:(sc + 1) * SC],
                        start=False, stop=True,
                    )
                return emit
            pend_v = make_last_v()
            pend_epi_a = make_epilogue_a(ci, b, sc, sc_ps)
            pend_epi_b = make_epilogue_b(ci, b, sc, vals_list)
            keysT_cur = keysT_next

        pend_v()
        pend_epi_a()
        pend_epi_b()

    nc.compile()
    return nc


def _get_nc():
    if "nc" not in _CACHE:
        _CACHE["nc"] = _build()
    return _CACHE["nc"]


def _make_in_maps(inputs):
    import ml_dtypes

    q = np.asarray(inputs["query"], dtype=np.float32)
    k = np.asarray(inputs["keys"], dtype=np.float32)
    val = np.asarray(inputs["values"], dtype=np.float32)
    m = np.asarray(inputs["mask"], dtype=np.int32)
    w1 = np.ascontiguousarray(np.asarray(inputs["W1"], dtype=np.float32))
    w2 = np.ascontiguousarray(np.asarray(inputs["W2"], dtype=np.float32))
    v = np.ascontiguousarray(np.asarray(inputs["v"], dtype=np.float32))
    ident = np.eye(128, dtype=ml_dtypes.bfloat16)
    identf = np.eye(8, dtype=np.float32)

    in_maps = []
    for c in range(NCORES):
        sl = slice(c * B_PC, (c + 1) * B_PC)
        in_maps.append({
            "keys": np.ascontiguousarray(k[sl]),
            "values": np.ascontiguousarray(val[sl]),
            "query": np.ascontiguousarray(q[sl]),
            "mask": np.ascontiguousarray(m[sl]),
            "W1": w1, "W2": w2, "v": v, "ident": ident, "identf": identf,
        })
    return in_maps


def kernel(**inputs):
    import time
    from concourse.bass_utils import run_bass_kernel_spmd

    nc = _get_nc()
    in_maps = _make_in_maps(inputs)
    last_err = None
    for attempt in range(3):
        try:
            res = run_bass_kernel_spmd(nc, in_maps, list(range(NCORES)))
            break
        except Exception as e:  # transient device errors: retry
            last_err = e
            time.sleep(2.0)
    else:
        raise last_err
    ctx = np.concatenate([res.results[c]["out_ctx"] for c in range(NCORES)], axis=0)
    attn = np.concatenate([res.results[c]["out_attn"] for c in range(NCORES)], axis=0)
    return ctx, attn
